# revision 1
# baseline (speedup 1.0000x reference)
"""YOLO-style detection head decode on 8 Trainium2 NeuronCores.

Input : x [64, 255, 52, 52] f32
Output: [64, 8112, 85] f32  (bbox(4) | conf(1) | cls(80), sigmoid/exp decoded)

Strategy (pure data parallel, 8 batches per core):
  - host packs per-(batch,anchor) slabs [87, 2706] (grid padded 2704->2706):
      rows 0..82 = channels [tx, ty, conf, cls0..79]  (tw/th dropped)
      rows 83/84 = stride*cx / stride*cy grid constants
      rows 85/86 = raw tw / th (read only by the exp staging pass)
    The per-slab device load covers rows 0..84 in one contiguous DMA.
  - ACT sigmoid on rows 0..82.
  - exp(tw/th + ln(anchor_px)) for all 24 slabs packed into ONE [48, 2706]
    ACT Exp op (anchor scale folded into the bias, raw rows fetched by a
    single 4-level-AP DMA), distributed back into slab rows 85/86 by
    SBUF->SBUF DMA on the otherwise-idle gpsimd engine.
  - transpose to output layout fused into PE matmuls: lhsT = 87 slab rows,
    rhs = constant [87, 85] matrix (stride scale on tx/ty, cx/cy rows ->
    cols 0/1, exp rows -> cols 2/3, data rows -> cols 4..84).  lhsT free dim
    strided by 22 so each of 123 output partitions holds 22 consecutive
    output rows -> 7480 B contiguous DMA store runs.
  - PSUM drained via 3 wide copies/slab (DVE/ACT/DVE) from 2-bank psum tiles.
"""

import numpy as np

G = 52
GG = G * G  # 2704
A = 3
NCH = 85  # 5 + 80
B = 64
N_CORES = 8
B_PER_CORE = B // N_CORES  # 8
STRIDE = 8.0  # 416 / 52
ANCHORS_PX = np.array([[10.0, 13.0], [16.0, 30.0], [33.0, 23.0]], dtype=np.float32)
K_MM = 87  # 83 sigmoid rows + 2 cxcy + 2 exp
R = 22  # output rows per partition
P_OUT = 123  # output partitions per matmul (123*22 = 2706 >= 2704)
FREE = P_OUT * R  # 2706
N_SLABS = B_PER_CORE * A  # 24

_CACHE = {}

# channel order of the 83 sigmoid rows: tx, ty, conf, cls0..cls79
DATA_CH = np.array([0, 1] + list(range(4, NCH)), dtype=np.int64)


def _build_consts():
    g = np.arange(GG, dtype=np.float32)
    cx = STRIDE * (g % G)
    cy = STRIDE * (g // G)
    cxcy = np.zeros((2, FREE), dtype=np.float32)
    cxcy[0, :GG] = cx
    cxcy[1, :GG] = cy

    mmat = np.zeros((K_MM, NCH), dtype=np.float32)
    mmat[0, 0] = STRIDE  # sigmoid(tx) -> col 0, scaled
    mmat[1, 1] = STRIDE  # sigmoid(ty) -> col 1, scaled
    for k in range(81):  # conf + cls -> cols 4..84
        mmat[2 + k, 4 + k] = 1.0
    mmat[83, 0] = 1.0  # stride*cx row -> col 0
    mmat[84, 1] = 1.0  # stride*cy row -> col 1
    mmat[85, 2] = 1.0  # exp(tw)*aw row -> col 2
    mmat[86, 3] = 1.0  # exp(th)*ah row -> col 3

    ebias = np.zeros((2 * N_SLABS, 1), dtype=np.float32)
    for b in range(B_PER_CORE):
        for a in range(A):
            s = A * b + a
            ebias[2 * s + 0, 0] = np.log(ANCHORS_PX[a, 0])
            ebias[2 * s + 1, 0] = np.log(ANCHORS_PX[a, 1])
    return cxcy, mmat, ebias


def build_nc():
    if "nc" in _CACHE:
        return _CACHE["nc"]
    from contextlib import ExitStack

    import concourse.bacc as bacc
    import concourse.tile as tile
    from concourse import mybir
    from concourse.tile_rust import add_dep_helper

    AF = mybir.ActivationFunctionType
    dt = mybir.dt

    nc = bacc.Bacc("TRN2", target_bir_lowering=False, debug=False)
    xe_t = nc.dram_tensor(
        "xe", [B_PER_CORE, A, K_MM, FREE], dt.float32, kind="ExternalInput"
    )
    mmat_t = nc.dram_tensor("mmat", [K_MM, NCH], dt.float32, kind="ExternalInput")
    ebias_t = nc.dram_tensor(
        "ebias", [2 * N_SLABS, 1], dt.float32, kind="ExternalInput"
    )
    out_t = nc.dram_tensor(
        "out", [B_PER_CORE, A, GG, NCH], dt.float32, kind="ExternalOutput"
    )
    xe_ap = xe_t.ap()
    mmat_ap = mmat_t.ap()
    ebias_ap = ebias_t.ap()
    out_ap = out_t.ap()

    with ExitStack() as ctx:
        tc = ctx.enter_context(tile.TileContext(nc))
        singles = ctx.enter_context(tc.tile_pool(name="singles", bufs=1))
        slabs = ctx.enter_context(tc.tile_pool(name="slabs", bufs=4))
        outs = ctx.enter_context(tc.tile_pool(name="outs", bufs=4))
        psums = ctx.enter_context(tc.tile_pool(name="psum", bufs=4, space="PSUM"))

        # preload both ACT LUT table sets at t~0 (sigmoid first, exp last so
        # the real exp below needs no reload); otherwise the first table load
        # serializes in front of the exp->sigmoid chain during pipeline fill
        dummy = singles.tile([1, 2], dt.float32)
        nc.vector.memset(dummy[:, :], 0.0)
        nc.scalar.activation(dummy[:, 1:2], dummy[:, 1:2], AF.Sigmoid)
        nc.scalar.activation(dummy[:, 0:1], dummy[:, 0:1], AF.Exp)

        # one 4-level-AP DMA loads every slab's raw tw/th rows at once, so
        # the exp op (and the first slab loads behind it on the SP sequencer)
        # aren't gated on a chain of small transfers
        staging = singles.tile([2 * N_SLABS, FREE], dt.float32)
        ebias_sb = singles.tile([2 * N_SLABS, 1], dt.float32)
        mmat_sb = singles.tile([K_MM, NCH], dt.float32)
        nc.sync.dma_start(out=staging[:, :], in_=xe_ap[:, :, 85:87, :])
        nc.sync.dma_start(out=ebias_sb[:, :], in_=ebias_ap[:, :])
        nc.sync.dma_start(out=mmat_sb[:, :], in_=mmat_ap[:, :])
        nc.scalar.activation(
            staging[:, :], staging[:, :], AF.Exp, bias=ebias_sb[:, :]
        )

        # warm the PE (HAM + pipeline) with throwaway matmuls on the constant
        # matrix while the first slab loads stream in
        wps = psums.tile([P_OUT, 2, 512], dt.float32, tag="ps")
        for _ in range(16):
            nc.tensor.matmul(
                wps[0:NCH, 0, 0:NCH], mmat_sb[:, :], mmat_sb[:, :],
                start=True, stop=True,
            )

        s2s0 = None
        for b in range(B_PER_CORE):
            for a in range(A):
                s = A * b + a
                slab = slabs.tile([K_MM, FREE], dt.float32)
                # exp rows move by DMA (engine copies need 32-aligned partition
                # bases); issued before the load so the transfer isn't queued
                # behind it
                s2s_i = nc.gpsimd.dma_start(
                    out=slab[85:87, :], in_=staging[2 * s : 2 * s + 2, :]
                )
                if s == 0:
                    s2s0 = s2s_i
                load_i = nc.sync.dma_start(
                    out=slab[0:85, :], in_=xe_ap[b, a, 0:85, :]
                )
                if s == 2:
                    # during pipeline fill, let slab0's tiny exp-row transfer
                    # reach the DMA engines before this load occupies them --
                    # otherwise slab0's matmuls (and the whole slab-buffer
                    # rotation behind them) wait for 4 queued 2.5us loads
                    add_dep_helper(
                        load_i.ins, s2s0.ins, sync=True,
                        reason="pipeline fill: exp-row transfer before load2",
                    )
                nc.scalar.activation(slab[0:83, :], slab[0:83, :], AF.Sigmoid)
                # [K_MM, P_OUT, R]: free index (p, t) -> grid row R*p + t
                slab_r = slab[:, :].rearrange("k (p t) -> k p t", t=R)

                out_sb = outs.tile([P_OUT, R * NCH], dt.float32)
                for pair in range(2):
                    ps = psums.tile([P_OUT, 2, 512], dt.float32, tag="ps")
                    for j in range(12):
                        t = 12 * pair + j
                        if t >= R:
                            break
                        # full 123 partitions even for t>=20: pad cols of xe
                        # are zero, so the 2 out-of-range grid rows compute
                        # to benign zeros (excluded from the store DMAs)
                        bank, jj = divmod(j, 6)
                        nc.tensor.matmul(
                            ps[:, bank, jj * NCH : (jj + 1) * NCH],
                            slab_r[:, :, t],
                            mmat_sb[:, :],
                            start=True,
                            stop=True,
                        )
                    if pair == 0:
                        # t0..11 -> cols 0:1020 in one 2-level-AP copy
                        nc.vector.tensor_copy(
                            out_sb[:, 0 : 12 * NCH].rearrange(
                                "p (k c) -> p k c", k=2
                            ),
                            ps[:, :, 0 : 6 * NCH],
                        )
                    else:
                        nc.vector.tensor_copy(
                            out_sb[:, 12 * NCH : 18 * NCH], ps[:, 0, 0 : 6 * NCH]
                        )
                        # last copy on ACT: in-order with the store DMA below,
                        # so the store issues with no cross-engine wait
                        nc.scalar.copy(
                            out_sb[:, 18 * NCH : 22 * NCH], ps[:, 1, 0 : 4 * NCH]
                        )
                full = (P_OUT - 1) * R  # 2684 rows with a full partition
                if s >= N_SLABS - 2:
                    # split the last slabs' stores so the first column group
                    # ships as soon as its copy lands -> shorter drain tail
                    fr = out_ap[b, a, 0:full, :].rearrange(
                        "(p r) c -> p (r c)", r=R
                    )
                    nc.scalar.dma_start(
                        out=fr[:, 0 : 12 * NCH], in_=out_sb[0 : P_OUT - 1, 0 : 12 * NCH]
                    )
                    nc.scalar.dma_start(
                        out=fr[:, 12 * NCH :], in_=out_sb[0 : P_OUT - 1, 12 * NCH :]
                    )
                else:
                    nc.scalar.dma_start(
                        out=out_ap[b, a, 0:full, :],
                        in_=out_sb[0 : P_OUT - 1, :],
                    )
                nc.scalar.dma_start(
                    out=out_ap[b, a, full:GG, :],
                    in_=out_sb[P_OUT - 1 : P_OUT, 0 : (GG - full) * NCH],
                )

    nc.compile()
    _CACHE["nc"] = nc
    return nc


def _pack_core_input(x_core):
    """x_core [B_PER_CORE, 255, 52, 52] -> xe [B_PER_CORE, A, 87, FREE]."""
    cxcy, _, _ = _build_consts()
    xr = x_core.reshape(B_PER_CORE, A, NCH, GG)
    xe = np.zeros((B_PER_CORE, A, K_MM, FREE), dtype=np.float32)
    xe[:, :, 0:83, 0:GG] = xr[:, :, DATA_CH, :]
    xe[:, :, 83:85, :] = cxcy[None, None]
    xe[:, :, 85:87, 0:GG] = xr[:, :, 2:4, :]
    return xe


def kernel(x):
    x = np.ascontiguousarray(np.asarray(x), dtype=np.float32)
    assert x.shape == (B, A * NCH, G, G), x.shape
    nc = build_nc()
    from concourse.bass_utils import run_bass_kernel_spmd

    _, mmat, ebias = _build_consts()
    in_maps = []
    for c in range(N_CORES):
        in_maps.append(
            {
                "xe": _pack_core_input(x[c * B_PER_CORE : (c + 1) * B_PER_CORE]),
                "mmat": mmat,
                "ebias": ebias,
            }
        )
    # transient NRT_EXEC_UNIT_UNRECOVERABLE has been observed once on a cold
    # first execution and never again; retry a couple of times before failing
    for attempt in range(3):
        try:
            res = run_bass_kernel_spmd(nc, in_maps, core_ids=list(range(N_CORES)))
            break
        except Exception:  # noqa: BLE001
            if attempt == 2:
                raise
            import time

            time.sleep(2.0 * (attempt + 1))
    _CACHE["last_res"] = res
    out = np.concatenate([r["out"] for r in res.results], axis=0)
    return out.reshape(B, A * GG, NCH)



# revision 5
# speedup vs baseline: 1.6099x; 1.6099x over previous
"""YOLO-style detection head decode on 8 Trainium2 NeuronCores.

Input : x [64, 255, 52, 52] f32
Output: [64, 8112, 85] f32  (bbox(4) | conf(1) | cls(80), sigmoid/exp decoded)

The kernel is DMA-bound (360 B/ns shared DMA engines), so everything rides
the wire as fp16 (rel err ~2e-3 vs the 2e-2 gate; bf16's 7-bit mantissa
fails at sigmoid tails).  Per core: 8 batches x 3 anchors = 24 slabs.

  - host packs xe [8, 3, 87, 2706] fp16 per slab (grid padded 2704->2706):
      rows 0..3  = raw tx, ty, tw, th
      rows 4..84 = raw conf, cls0..79      (sigmoid applied POST-transpose)
      rows 85/86 = 8*cx / 8*cy grid constants (exact in fp16)
  - tx/ty sigmoid + tw/th exp(+ln anchor bias) run as two single [48, 2706]
    ACT ops on staging tiles (one op for all 24 slabs; a per-slab [2, 2706]
    op would cost the same 2.3us each on ACT's free-dim clock), then move
    into slab rows 0:4 by SBUF->SBUF DMA on the otherwise-idle Pool engine.
  - per slab: one 83-row HBM load (rows 4:87), 22 fp16 matmuls (+2 dummies
    to fill the 24-group psum) transpose to output layout: lhsT free dim
    strided by 22 so each of 123 output partitions holds 22 consecutive
    output rows; rhs = constant [87, 85] matrix (8x scale on tx/ty, grid
    rows -> cols 0/1, exp rows -> cols 2/3, raw rows -> cols 4..84).
  - psum [123, 4, 512] f32 (4 banks, 6 x 85-col groups per bank) drains via
    ONE whole-tile ACT sigmoid [123, (4, 510)] f32->fp16 — decoding conf/cls
    while transposing — then 4 small DVE copies re-overwrite bbox cols 0:4
    of each group with the raw psum values (already decoded pre-matmul).
  - out DRAM padded to [.., 2706, 85] so each slab stores with a single
    uniform [123 x 3740B] DMA; host drops the 2 pad rows when gathering.
"""

import numpy as np

G = 52
GG = G * G  # 2704
A = 3
NCH = 85  # 5 + 80
B = 64
N_CORES = 8
B_PER_CORE = B // N_CORES  # 8
STRIDE = 8.0  # 416 / 52
ANCHORS_PX = np.array([[10.0, 13.0], [16.0, 30.0], [33.0, 23.0]], dtype=np.float32)
K_MM = 87  # 4 decoded + 81 raw + 2 grid rows
R = 22  # output rows per partition
P_OUT = 123  # output partitions per matmul (123*22 = 2706 >= 2704)
FREE = P_OUT * R  # 2706
N_SLABS = B_PER_CORE * A  # 24
NGRP = 24  # psum groups per slab (22 real + 2 dummy)
OUT_COLS = NGRP * NCH  # 2040

_CACHE = {}


def _build_consts():
    mmat = np.zeros((K_MM, NCH), dtype=np.float16)
    mmat[0, 0] = STRIDE  # sigmoid(tx) -> col 0, x8
    mmat[1, 1] = STRIDE  # sigmoid(ty) -> col 1, x8
    mmat[2, 2] = 1.0  # exp(tw)*aw_px -> col 2
    mmat[3, 3] = 1.0  # exp(th)*ah_px -> col 3
    for k in range(81):  # raw conf + cls -> cols 4..84
        mmat[4 + k, 4 + k] = 1.0
    mmat[85, 0] = 1.0  # 8*cx row -> col 0
    mmat[86, 1] = 1.0  # 8*cy row -> col 1

    ebias = np.zeros((2 * N_SLABS, 1), dtype=np.float32)
    for b in range(B_PER_CORE):
        for a in range(A):
            s = A * b + a
            ebias[2 * s + 0, 0] = np.log(ANCHORS_PX[a, 0])
            ebias[2 * s + 1, 0] = np.log(ANCHORS_PX[a, 1])

    g = np.arange(GG, dtype=np.float32)
    cxcy = np.zeros((2, FREE), dtype=np.float16)
    cxcy[0, :GG] = (STRIDE * (g % G)).astype(np.float16)  # multiples of 8: exact
    cxcy[1, :GG] = (STRIDE * (g // G)).astype(np.float16)
    return mmat, ebias, cxcy


def build_nc():
    if "nc" in _CACHE:
        return _CACHE["nc"]
    from contextlib import ExitStack

    import concourse.bacc as bacc
    import concourse.tile as tile
    from concourse import mybir

    AF = mybir.ActivationFunctionType
    dt = mybir.dt

    nc = bacc.Bacc("TRN2", target_bir_lowering=False, debug=False)
    xe_t = nc.dram_tensor(
        "xe", [B_PER_CORE, A, K_MM, FREE], dt.float16, kind="ExternalInput"
    )
    mmat_t = nc.dram_tensor("mmat", [K_MM, NCH], dt.float16, kind="ExternalInput")
    ebias_t = nc.dram_tensor(
        "ebias", [2 * N_SLABS, 1], dt.float32, kind="ExternalInput"
    )
    out_t = nc.dram_tensor(
        "out", [B_PER_CORE, A, FREE, NCH], dt.float16, kind="ExternalOutput"
    )
    xe_ap = xe_t.ap()
    mmat_ap = mmat_t.ap()
    ebias_ap = ebias_t.ap()
    out_ap = out_t.ap()

    with ExitStack() as ctx:
        tc = ctx.enter_context(tile.TileContext(nc))
        singles = ctx.enter_context(tc.tile_pool(name="singles", bufs=1))
        slabs = ctx.enter_context(tc.tile_pool(name="slabs", bufs=4))
        outs = ctx.enter_context(tc.tile_pool(name="outs", bufs=4))
        psums = ctx.enter_context(tc.tile_pool(name="psum", bufs=2, space="PSUM"))

        stg_sig = singles.tile([2 * N_SLABS, FREE], dt.float16)
        stg_exp = singles.tile([2 * N_SLABS, FREE], dt.float16)
        ebias_sb = singles.tile([2 * N_SLABS, 1], dt.float32)
        mmat_sb = singles.tile([K_MM, NCH], dt.float16)

        # table preloads at t~0 (sigmoid first: the staged sigmoid then needs
        # no reload; the sigmoid reload after the staged exp hides behind the
        # first slab's load+matmul latency)
        dummy = singles.tile([1, 2], dt.float16)
        nc.vector.memset(dummy[:, :], 0.0)
        nc.scalar.activation(dummy[:, 0:1], dummy[:, 0:1], AF.Sigmoid)
        nc.scalar.activation(dummy[:, 1:2], dummy[:, 1:2], AF.Exp)

        # staged decode of tx/ty (sigmoid) and tw/th (exp with ln-anchor
        # bias), one [48, 2706] op each for all 24 slabs
        nc.sync.dma_start(out=stg_sig[:, :], in_=xe_ap[:, :, 0:2, :])
        nc.sync.dma_start(out=stg_exp[:, :], in_=xe_ap[:, :, 2:4, :])
        nc.sync.dma_start(out=ebias_sb[:, :], in_=ebias_ap[:, :])
        nc.sync.dma_start(out=mmat_sb[:, :], in_=mmat_ap[:, :])
        nc.scalar.activation(stg_sig[:, :], stg_sig[:, :], AF.Sigmoid)
        nc.scalar.activation(
            stg_exp[:, :], stg_exp[:, :], AF.Exp, bias=ebias_sb[:, :]
        )

        # warm the PE (pipeline + p-state) on the constant matrix while the
        # first slab loads stream in
        wps = psums.tile([P_OUT, 4, 512], dt.float32, tag="ps")
        for _ in range(16):
            nc.tensor.matmul(
                wps[0:NCH, 0, 0:NCH], mmat_sb[:, :], mmat_sb[:, :],
                start=True, stop=True,
            )

        # stores issue from SP two slabs late so their semaphore waits never
        # block the next loads' issue (HWDGE here is SP/ACT only; ACT's
        # sequencer has no slack and Pool's SWDGE engine time is too dear)
        pending_stores = []

        def flush_store():
            bb, aa, sb_tile = pending_stores.pop(0)
            nc.sync.dma_start(
                out=out_ap[bb, aa, :, :].rearrange("(p r) c -> p (r c)", r=R),
                in_=sb_tile[:, 0 : R * NCH],
            )

        for b in range(B_PER_CORE):
            for a in range(A):
                s = A * b + a
                slab = slabs.tile([K_MM, FREE], dt.float16)
                # decoded rows move by DMA on the Pool engine (engine copies
                # need 32-aligned partition bases); issued before the load so
                # the transfers aren't queued behind it
                nc.gpsimd.dma_start(
                    out=slab[0:2, :], in_=stg_sig[2 * s : 2 * s + 2, :]
                )
                nc.gpsimd.dma_start(
                    out=slab[2:4, :], in_=stg_exp[2 * s : 2 * s + 2, :]
                )
                nc.sync.dma_start(out=slab[4:K_MM, :], in_=xe_ap[b, a, 4:K_MM, :])
                if len(pending_stores) >= 2:
                    flush_store()
                # [K_MM, P_OUT, R]: free index (p, t) -> grid row R*p + t
                slab_r = slab[:, :].rearrange("k (p t) -> k p t", t=R)

                ps = psums.tile([P_OUT, 4, 512], dt.float32, tag="ps")
                for k in range(NGRP):
                    # full 123 partitions even for t>=20: pad cols of xe are
                    # zero, so the 2 out-of-range grid rows compute to benign
                    # values (excluded from the store DMA); groups 22/23 are
                    # dummies that keep the psum tile uniformly initialized
                    # for the whole-tile sigmoid drain below
                    t = k if k < R else 0
                    bank, jj = divmod(k, 6)
                    nc.tensor.matmul(
                        ps[:, bank, jj * NCH : (jj + 1) * NCH],
                        slab_r[:, :, t],
                        mmat_sb[:, :],
                        start=True,
                        stop=True,
                    )

                out_sb = outs.tile([P_OUT, OUT_COLS], dt.float16)
                # one whole-tile drain decodes conf/cls (cols 4:85 of every
                # group) while converting f32->fp16; bbox cols get sigmoid'd
                # garbage here and are re-overwritten from psum just below
                nc.scalar.activation(
                    out_sb[:, :].rearrange("p (b c) -> p b c", c=6 * NCH),
                    ps[:, :, 0 : 6 * NCH],
                    AF.Sigmoid,
                )
                for bank in range(4):
                    nc.vector.tensor_copy(
                        out_sb[:, bank * 6 * NCH : (bank + 1) * 6 * NCH]
                        .rearrange("p (g c) -> p g c", c=NCH)[:, :, 0:4],
                        ps[:, bank, 0 : 6 * NCH]
                        .rearrange("p (g c) -> p g c", c=NCH)[:, :, 0:4],
                    )
                # uniform [123 x 3740B] store; DRAM rows 2704:2706 are pad
                pending_stores.append((b, a, out_sb))
        while pending_stores:
            flush_store()

    nc.compile()
    _CACHE["nc"] = nc
    return nc


# channel order of the raw rows 4..84: conf, cls0..79  (channels 4..84)
def _pack_core_input(x_core, cxcy):
    """x_core [B_PER_CORE, 255, 52, 52] f32 -> xe [B_PER_CORE, A, 87, FREE] f16."""
    xr = x_core.reshape(B_PER_CORE, A, NCH, GG)
    xe = np.zeros((B_PER_CORE, A, K_MM, FREE), dtype=np.float16)
    xe[:, :, 0:NCH, 0:GG] = xr
    xe[:, :, NCH : K_MM, :] = cxcy[None, None]
    return xe


def kernel(x):
    x = np.ascontiguousarray(np.asarray(x), dtype=np.float32)
    assert x.shape == (B, A * NCH, G, G), x.shape
    nc = build_nc()
    from concourse.bass_utils import run_bass_kernel_spmd

    mmat, ebias, cxcy = _build_consts()
    in_maps = []
    for c in range(N_CORES):
        in_maps.append(
            {
                "xe": _pack_core_input(x[c * B_PER_CORE : (c + 1) * B_PER_CORE], cxcy),
                "mmat": mmat,
                "ebias": ebias,
            }
        )
    # transient NRT_EXEC_UNIT_UNRECOVERABLE has been observed once on a cold
    # first execution and never again; retry a couple of times before failing
    for attempt in range(3):
        try:
            res = run_bass_kernel_spmd(nc, in_maps, core_ids=list(range(N_CORES)))
            break
        except Exception:  # noqa: BLE001
            if attempt == 2:
                raise
            import time

            time.sleep(2.0 * (attempt + 1))
    _CACHE["last_res"] = res
    out = np.concatenate([r["out"] for r in res.results], axis=0)
    out = out[:, :, 0:GG, :].astype(np.float32)
    return out.reshape(B, A * GG, NCH)
